# revision 1
# baseline (speedup 1.0000x reference)
"""Trainium2 Bass kernel for nn_BondDecoder (histogram_binning).

Math (derived exactly from the reference):
  a_i = 1 - src_mask ; t_i = tgt_mask ; c = a*t
  loss_b = sum_ij (a_i a_j - c_i c_j) * z_ij^2
  z = sum_h softmax_inc_h - sum_h softmax_dec_h + H_src - (g_i g_j) H_tgt

Every term carries a_i * a_j, so only unmasked (src) tokens matter. Host
compacts tokens to the first n_b positions and pads to J (=288 covers
n_b<=276 with margin; auto-rebuilds at larger J if ever exceeded). This
removes the key-mask entirely: padded k columns are exactly zero, so
padded scores are exactly 0, exp gives exactly 1, and the softmax row sum
is corrected by the host-provided constant -(J - n_b).

Device pipeline per core (4 batch elements):
  - projections as fp8(e4m3) DoubleRow matmuls (2x PE rate), folded
    conv1d+inproj weights pre-scaled by 32 so fp8 entries sit in the
    normal range; the 1/32^2 is folded into the exp scale.
  - per-head QK^T scores in fp16 into a 4-bank PSUM group; one mega-exp
    activation per 4-head group (amortizes ACT fixed overhead).
  - row sums via DVE tensor_scalar accum_out (4x mode), reciprocal once
    per [128,8] block; dec-head weights negated so z accumulates with
    adds only.
  - z assembled on PE: diag(w_g) matmuls accumulate normalized heads
    into PSUM on top of an I @ D seed; diag tiles built on DVE as
    (+-identity * w) — dec heads use -I so no separate negation op.
  - Square on ACT, then quadratic forms  u^T zsq u  and  c^T zsq c  on
    PE, final dot+reduce on DVE.
"""

from contextlib import ExitStack

import numpy as np

import concourse.bacc as bacc
import concourse.mybir as mybir
import concourse.tile as tile
from concourse.bass_utils import run_bass_kernel_spmd

L = 512
B = 32
D = 512
NCORES = 8
BPC = B // NCORES  # batch elements per core
NH = 4
HD = D // NH  # 128
JDEF = 288  # compacted+padded token count (seed-0 max n_b = 276)
S8 = 32.0  # fp8 pre-scale on folded projection weights
SCALE = float(1.0 / np.sqrt(HD) / (S8 * S8))

F8 = mybir.dt.float8e4
F16 = mybir.dt.float16
F32 = mybir.dt.float32
AF = mybir.ActivationFunctionType
ALU = mybir.AluOpType
DR = mybir.MatmulPerfMode.DoubleRow

# which proj-psum pair-drains run on ACT (rest on DVE) — load balance knob
ACT_DRAINS = frozenset((1, 3, 5))
# build diag(w) tiles on gpsimd (Pool) or DVE. Pool is catastrophically
# slow on real HW (~1.6us per small op, 96 ops = +150us measured) despite
# the cost model's 254ns — never put small ops there.
DIAG_ON_POOL = False
# z*z on ACT (Square) or DVE (tensor_tensor mult)
SQ_ON_DVE = False
# drain proj PSUM in bank pairs (one 2J copy, pproj bufs=1) vs single
# banks (two J copies, pproj bufs=2). Pairing measured WORSE on HW
# (67-84us vs 44-72us): bufs=1 stalls PE behind each drain.
PAIRED_DRAINS = False
# emit all 4 batch elements' proj, then all phase-1s, then all phase-2s
# (deeper cross-b pipelining) instead of b-at-a-time
BATCH_B = True
# pack narrow-chunk heads two-per-bank so one mega-exp covers all 8 heads
PACK_HEADS = True

_CACHE = {}


def _chunks(J):
    out = []
    i0 = 0
    while i0 < J:
        out.append((i0, min(128, J - i0)))
        i0 += 128
    return out


def _emit(ctx, tc, dram, out_ap, J, repeat=1):
    nc = tc.nc
    ics = _chunks(J)
    nic = len(ics)

    # per-tag buffer depths: BATCH_B keeps all 4 batch elements' tiles
    # live through each phase, so lifetimes are ~4x longer
    if BATCH_B:
        QK_B, E_B, DG_B, D_B, PB_B = 6, 13, 13, 5, 5
    else:
        QK_B, E_B, DG_B, D_B, PB_B = 2, 4, 4, 2, 4

    const_pool = ctx.enter_context(tc.tile_pool(name="const", bufs=1))
    xt_pool = ctx.enter_context(tc.tile_pool(name="xt", bufs=3))
    qk_pool = ctx.enter_context(tc.tile_pool(name="qk", bufs=QK_B))
    e_pool = ctx.enter_context(tc.tile_pool(name="e", bufs=E_B))
    z_pool = ctx.enter_context(tc.tile_pool(name="z", bufs=3))
    dg_pool = ctx.enter_context(tc.tile_pool(name="dg", bufs=DG_B))
    small_pool = ctx.enter_context(tc.tile_pool(name="small", bufs=PB_B))
    psum_proj = ctx.enter_context(
        tc.tile_pool(name="pproj", bufs=1 if PAIRED_DRAINS else 2, space="PSUM")
    )
    psum_s = ctx.enter_context(tc.tile_pool(name="pscore", bufs=1, space="PSUM"))
    psum_z = ctx.enter_context(tc.tile_pool(name="pz", bufs=1, space="PSUM"))
    psum_q = ctx.enter_context(tc.tile_pool(name="pquad", bufs=1, space="PSUM"))

    # constants / parameters
    acat_t = []
    for e in range(2):
        t = const_pool.tile([128, 2, 4 * D], F8, tag=f"acat{e}")
        nc.sync.dma_start(t[:], dram["acat"][e])
        acat_t.append(t)
    i_t = const_pool.tile([128, 128], F16, tag="ident")
    nc.sync.dma_start(i_t[:], dram["ident"][:])
    ni_t = const_pool.tile([128, 128], F16, tag="nident")
    nc.sync.dma_start(ni_t[:], dram["nident"][:])

    def emit_loads(b):
        st = {"b": b}
        st["xt"] = xt_pool.tile([128, 2, 2, J], F8, tag="xt", name="xt")
        nc.sync.dma_start(st["xt"][:], dram["xt"][b])
        st["d"] = z_pool.tile([128, nic, J], F16, tag="dmat", bufs=D_B, name="dmat")
        nc.sync.dma_start(st["d"][:], dram["dmat"][b])
        st["uc"] = small_pool.tile([128, 2 * nic], F16, tag="uc", name="uc")
        nc.sync.dma_start(st["uc"][:], dram["uc"][b])
        st["acr"] = small_pool.tile([2, J], F32, tag="acr", name="acr")
        nc.sync.dma_start(st["acr"][:], dram["acr"][b])
        st["cn"] = small_pool.tile([128, 1], F32, tag="cn", name="cn")
        nc.sync.dma_start(st["cn"][:], dram["cn"][b])
        return st

    def emit_proj(st):
        # fp8 DoubleRow projections, K=512 as 2 chained K=256. The in-proj
        # q bias is dropped: it shifts scores by ~1e-2 nats (rel loss err
        # ~8e-5 in simulation), far under tolerance — so the drains are
        # plain copies, alternating ACT/DVE. k-side bias cancels exactly.
        qk = []
        if PAIRED_DRAINS:
            for pair in range(8):
                ps = psum_proj.tile([128, 2, 512], F32, tag="pproj")
                for half in range(2):
                    dc = 2 * pair + half
                    for e in range(2):
                        nc.tensor.matmul(
                            ps[:, half, :J],
                            acat_t[e][:, :, 128 * dc : 128 * (dc + 1)],
                            st["xt"][:, e],
                            start=(e == 0),
                            stop=(e == 1),
                            perf_mode=DR,
                        )
                t = qk_pool.tile([128, 2, J], F16, tag=f"qk{pair}")
                if pair in ACT_DRAINS:
                    nc.scalar.activation(t[:], ps[:, :, :J], AF.Copy)
                else:
                    nc.vector.tensor_scalar_add(t[:], ps[:, :, :J], 0.0)
                qk.append(t[:, 0, :])
                qk.append(t[:, 1, :])
        else:
            for dc in range(16):
                ps = psum_proj.tile([128, 512], F32, tag="pproj")
                for e in range(2):
                    nc.tensor.matmul(
                        ps[:, :J],
                        acat_t[e][:, :, 128 * dc : 128 * (dc + 1)],
                        st["xt"][:, e],
                        start=(e == 0),
                        stop=(e == 1),
                        perf_mode=DR,
                    )
                t = qk_pool.tile([128, J], F16, tag=f"qk{dc}")
                if dc % 2 == 1:
                    nc.scalar.activation(t[:], ps[:, :J], AF.Copy)
                else:
                    nc.vector.tensor_scalar_add(t[:], ps[:, :J], 0.0)
                qk.append(t[:])
        st["qk"] = qk

    def emit_phase1(st):
        # scores, mega-exp, row sums, diag weights per i-chunk. Phase 2
        # is emitted separately so ACT's in-order queue runs exps
        # back-to-back instead of stalling on each ic's z chain.
        qk = st["qk"]
        diag_eng = nc.gpsimd if DIAG_ON_POOL else nc.vector
        st["Es"], st["dgss"], st["emap"] = [], [], []
        for ic, (i0, pp) in enumerate(ics):
            # heads are packed into PSUM banks at partition offsets when
            # the chunk is narrow (pp<=64): fewer banks -> fewer mega-exp
            # instructions (one 2-bank exp covers all 8 heads at pp=32)
            # PE out base partition must be in {0, 32, 64}, so at most
            # 2 partition-groups per bank (po in {0, pp})
            nbank = 4 if (PACK_HEADS and pp <= 64) else 8
            hpb = 8 // nbank  # partition-groups per bank
            emap = [((g % nbank) if nbank <= 4 else (g % 4),
                     ((g // nbank) * pp) if nbank <= 4 else 0)
                    for g in range(8)]
            st["emap"].append(emap)
            E = e_pool.tile([128, 8, J], F16, tag="E")
            rs = small_pool.tile([128, 8], F32, tag="rs", bufs=4)
            ngrp = 2 if nbank == 8 else 1
            for grp in range(ngrp):
                sc = psum_s.tile([128, 4, 512], F32, tag="pscore")
                gs = range(4 * grp, 4 * grp + 4) if ngrp == 2 else range(8)
                for g in gs:
                    qdc = (0 if g < 4 else 8) + (g % 4)
                    bk, po = emap[g]
                    nc.tensor.matmul(
                        sc[po : po + pp, bk, :J],
                        qk[qdc][:, i0 : i0 + pp],
                        qk[qdc + 4][:],
                        start=True,
                        stop=True,
                    )
                if ngrp == 2:
                    nc.scalar.activation(
                        E[:pp, 4 * grp : 4 * grp + 4, :],
                        sc[:pp, :, :J],
                        AF.Exp,
                        scale=SCALE,
                    )
                else:
                    nc.scalar.activation(
                        E[: hpb * pp, :nbank, :],
                        sc[: hpb * pp, :nbank, :J],
                        AF.Exp,
                        scale=SCALE,
                    )
            # row sums (DVE ts in 4x mode; reduce rides accum_out)
            scr = e_pool.tile([128, J], F16, tag="scratch", bufs=4)
            for g in range(8):
                bk, po = (emap[g] if ngrp == 1 else (g, 0))
                src_sl = E[po : po + pp, bk, :] if ngrp == 1 else E[:pp, g, :]
                nc.vector.tensor_scalar(
                    scr[:pp],
                    src_sl,
                    1.0,
                    0.0,
                    op0=ALU.mult,
                    op1=ALU.add,
                    accum_out=rs[po : po + pp, g : g + 1],
                )
            # pad-correct then reciprocal; dec-head negation is folded
            # into the diag build via +-I
            rs2 = small_pool.tile([128, 8], F32, tag="rs2", bufs=4)
            w = small_pool.tile([128, 8], F32, tag="w", bufs=4)
            if ngrp == 2:
                nc.vector.tensor_scalar(
                    rs2[:pp], rs[:pp], st["cn"][:pp], None, op0=ALU.add
                )
                nc.vector.reciprocal(w[:pp], rs2[:pp])
            else:
                for q in range(hpb):
                    sl = (slice(q * pp, (q + 1) * pp),
                          slice(q * nbank, (q + 1) * nbank))
                    nc.vector.tensor_scalar(
                        rs2[sl[0], sl[1]], rs[sl[0], sl[1]],
                        st["cn"][sl[0]], None, op0=ALU.add,
                    )
                    nc.vector.reciprocal(w[sl[0], sl[1]], rs2[sl[0], sl[1]])
            # diag(+-w_g): identity * per-row scalar. NEVER on gpsimd: Pool
            # costs ~1.6us per small op on real HW (+150us measured).
            dgs = []
            for g in range(8):
                bk, po = (emap[g] if ngrp == 1 else (g, 0))
                dg = dg_pool.tile([128, 128], F16, tag=f"diag{g}")
                diag_eng.tensor_scalar_mul(
                    dg[po : po + pp, :pp],
                    (i_t if g < 4 else ni_t)[po : po + pp, po : po + pp],
                    w[po : po + pp, g : g + 1],
                )
                dgs.append(dg)
            st["Es"].append(E)
            st["dgss"].append(dgs)

    def emit_phase2(st):
        # z = D + sum_g diag(w_g) @ E_g on PE, square, quad forms
        qf = psum_q.tile([2, 512], F32, tag="pquad")
        for ic, (i0, pp) in enumerate(ics):
            E, dgs, emap = st["Es"][ic], st["dgss"][ic], st["emap"][ic]
            packed = pp <= 64
            zp = psum_z.tile([128, 512], F32, tag="pz")
            nc.tensor.matmul(
                zp[:pp, :J], i_t[:pp, :pp], st["d"][:pp, ic], start=True, stop=False
            )
            for g in range(8):
                bk, po = (emap[g] if packed else (g, 0))
                e_sl = E[po : po + pp, bk, :] if packed else E[:pp, g, :]
                nc.tensor.matmul(
                    zp[:pp, :J],
                    dgs[g][po : po + pp, :pp],
                    e_sl,
                    start=False,
                    stop=(g == 7),
                )
            zq = z_pool.tile([128, J], F16, tag="zsq", bufs=3)
            if SQ_ON_DVE:
                nc.vector.tensor_mul(zq[:pp], zp[:pp, :J], zp[:pp, :J])
            else:
                nc.scalar.activation(zq[:pp], zp[:pp, :J], AF.Square)
            nc.tensor.matmul(
                qf[:, :J],
                st["uc"][:pp, 2 * ic : 2 * (ic + 1)],
                zq[:pp],
                start=(ic == 0),
                stop=(ic == nic - 1),
            )
        st["qf"] = qf

    def emit_tail(st):
        # final dots: sum_j (u^T W)_j u_j  and  -sum_j (c^T W)_j c_j
        fd = small_pool.tile([2, J], F32, tag="fd")
        red = small_pool.tile([2, 1], F32, tag="red")
        nc.vector.tensor_mul(fd[:], st["qf"][:, :J], st["acr"][:])
        nc.vector.tensor_reduce(red[:], fd[:], axis=mybir.AxisListType.X, op=ALU.add)
        nc.sync.dma_start(out_ap[st["b"]], red[:])

    if BATCH_B:
        for _ in range(repeat):
            sts = []
            for b in range(BPC):
                st = emit_loads(b)
                emit_proj(st)
                sts.append(st)
            for st in sts:
                emit_phase1(st)
            for st in sts:
                emit_phase2(st)
                emit_tail(st)
    else:
        # b's tail is deferred into the middle of b+1's emission: it
        # depends on b's full pipeline, so emitting it at the front of
        # b+1 would stall DVE's (and SP's) in-order queues.
        tail_st = None
        for b in [b for _ in range(repeat) for b in range(BPC)]:
            st = emit_loads(b)
            emit_proj(st)
            emit_phase1(st)
            if tail_st is not None:
                emit_tail(tail_st)
                tail_st = None
            emit_phase2(st)
            tail_st = st
        if tail_st is not None:
            emit_tail(tail_st)


def _build(J, repeat=1):
    nc = bacc.Bacc(
        "TRN2",
        target_bir_lowering=False,
        debug=False,
        num_devices=NCORES,
    )
    nic = len(_chunks(J))
    dram = {
        "acat": nc.dram_tensor("acat", [2, 128, 2, 4 * D], F8, kind="ExternalInput").ap(),
        "ident": nc.dram_tensor("ident", [128, 128], F16, kind="ExternalInput").ap(),
        "nident": nc.dram_tensor("nident", [128, 128], F16, kind="ExternalInput").ap(),
        "xt": nc.dram_tensor("xt", [BPC, 128, 2, 2, J], F8, kind="ExternalInput").ap(),
        "dmat": nc.dram_tensor("dmat", [BPC, 128, nic, J], F16, kind="ExternalInput").ap(),
        "uc": nc.dram_tensor("uc", [BPC, 128, 2 * nic], F16, kind="ExternalInput").ap(),
        "acr": nc.dram_tensor("acr", [BPC, 2, J], F32, kind="ExternalInput").ap(),
        "cn": nc.dram_tensor("cn", [BPC, 128, 1], F32, kind="ExternalInput").ap(),
    }
    out_ap = nc.dram_tensor("out", [BPC, 2], F32, kind="ExternalOutput").ap()
    with tile.TileContext(nc) as tc, ExitStack() as ctx:
        _emit(ctx, tc, dram, out_ap, J, repeat=repeat)
    nc.compile()
    return nc


def get_nc(J=JDEF, repeat=1):
    key = (J, repeat)
    if key not in _CACHE:
        _CACHE[key] = _build(J, repeat=repeat)
    return _CACHE[key]


def _fold(cw, cb, W, bb):
    # q = (x @ cw.T + cb) @ W.T + bb  ==  x @ A + bias
    A = (W.astype(np.float64) @ cw.astype(np.float64)).T
    bias = cb.astype(np.float64) @ W.astype(np.float64).T + bb
    return A.astype(np.float32), bias.astype(np.float32)


def prepare_in_maps(inputs, J=None):
    np8 = mybir.dt.np(F8)
    me = np.asarray(inputs["molecule_embedding"], np.float32)  # [L, B, D]
    src_bond = np.asarray(inputs["src_bond"]).astype(np.int64)  # [B, L, 6]
    tgt_bond = np.asarray(inputs["tgt_bond"]).astype(np.int64)
    src_mask = np.asarray(inputs["src_mask"]).astype(bool)  # [B, L]
    tgt_mask = np.asarray(inputs["tgt_mask"]).astype(bool)

    idxs = [np.where(~src_mask[b])[0] for b in range(B)]
    nmax = max(len(ix) for ix in idxs)
    if J is None:
        J = JDEF if nmax <= JDEF else 32 * ((nmax + 31) // 32)
    nic = len(_chunks(J))

    A_qi, b_qi = _fold(inputs["inc_q_w"], inputs["inc_q_b"], inputs["inc_Wq"], inputs["inc_bq"])
    A_ki, _ = _fold(inputs["inc_k_w"], inputs["inc_k_b"], inputs["inc_Wk"], inputs["inc_bk"])
    A_qd, b_qd = _fold(inputs["dec_q_w"], inputs["dec_q_b"], inputs["dec_Wq"], inputs["dec_bq"])
    A_kd, _ = _fold(inputs["dec_k_w"], inputs["dec_k_b"], inputs["dec_Wk"], inputs["dec_bk"])
    acat = np.concatenate([A_qi, A_ki, A_qd, A_kd], axis=1)  # [512, 2048]
    # DoubleRow pack: logical K row kappa = 256*e + 2*p + slot
    acat8 = (acat * S8).astype(np8).reshape(2, 128, 2, 4 * D)
    ident = np.eye(128, dtype=np.float16)
    nident = -ident

    t_all = tgt_mask.astype(np.float32)
    g_all = 1.0 - t_all

    # bond histograms -> D = H_src - (g_i g_j) H_tgt  (small exact integers)
    bi = np.arange(B)[:, None, None]
    li = np.arange(L)[None, :, None]
    H_s = np.zeros((B, L, L), np.float32)
    np.add.at(H_s, (bi, li, src_bond), 1.0)
    H_t = np.zeros((B, L, L), np.float32)
    np.add.at(H_t, (bi, li, tgt_bond), 1.0)
    D_full = H_s - g_all[:, :, None] * g_all[:, None, :] * H_t

    xt = np.zeros((B, 128, 2, 2, J), np8)
    dmat = np.zeros((B, 128, nic, J), np.float16)
    uc = np.zeros((B, 128, 2 * nic), np.float16)
    acr = np.zeros((B, 2, J), np.float32)
    cn = np.zeros((B, 128, 1), np.float32)
    for b in range(B):
        ix = idxs[b]
        n = len(ix)
        xp = np.zeros((D, J), np.float32)
        xp[:, :n] = me[ix, b, :].T  # [512, n]
        # DoubleRow pack kappa = 256*e + 2*p + i -> [p, e, i, j]
        xt[b] = xp.astype(np8).reshape(2, 128, 2, J).transpose(1, 0, 2, 3)
        dpad = np.zeros((128 * nic, J), np.float32)
        dpad[:n, :n] = D_full[b][np.ix_(ix, ix)]
        dmat[b] = dpad.reshape(nic, 128, J).transpose(1, 0, 2)
        u = np.zeros(J, np.float32)
        u[:n] = 1.0
        c = np.zeros(J, np.float32)
        c[:n] = t_all[b][ix]
        for ic in range(nic):
            seg = slice(128 * ic, min(128 * (ic + 1), J))
            m = seg.stop - seg.start
            uc[b, :m, 2 * ic] = u[seg]
            uc[b, :m, 2 * ic + 1] = c[seg]
        acr[b, 0] = u
        acr[b, 1] = -c
        cn[b, :, 0] = -(float(J - n))

    in_maps = []
    for cid in range(NCORES):
        sl = slice(cid * BPC, (cid + 1) * BPC)
        in_maps.append(
            {
                "acat": acat8,
                "ident": ident,
                "nident": nident,
                "xt": np.ascontiguousarray(xt[sl]),
                "dmat": np.ascontiguousarray(dmat[sl]),
                "uc": np.ascontiguousarray(uc[sl]),
                "acr": np.ascontiguousarray(acr[sl]),
                "cn": np.ascontiguousarray(cn[sl]),
            }
        )
    return in_maps, J, acr


def finish(results, acr):
    outp = np.concatenate([r["out"] for r in results], axis=0)  # [B, 2]
    return (outp[:, 0] + outp[:, 1]).astype(np.float32)


def kernel(**inputs):
    in_maps, J, acr = prepare_in_maps(inputs)
    nc = get_nc(J)
    res = run_bass_kernel_spmd(nc, in_maps, core_ids=list(range(NCORES)))
    return finish(res.results, acr)


if __name__ == "__main__":
    print("kernel module loaded OK")

